# revision 12
# baseline (speedup 1.0000x reference)
"""Trainium2 Bass kernel for nn_Discriminator_15668040696127.

Computes:
    q, a, d = samples[:, 0], samples[:, 1], samples[:, 2]        # [B, D]
    cos1 = <q,d> / max(||q||*||d||, 1e-6)                         # [B]
    cos2 = <a,d> / max(||a||*||d||, 1e-6)                         # [B]
    score = cos1 @ D_v1 + cos2 @ D_v2                             # scalar
    out = BCE_with_logits(score, labels[0])                       # scalar

Sharding: data-parallel over B across 8 NeuronCores (1024 samples each).
Each core computes a partial score, broadcast into an [8,1] tile; an
AllToAll (Mesh, ~13us vs 23us for the RDH ReduceScatter) hands every
rank all 8 partials, which it sums locally before the scalar BCE; the
host reads core 0's output.

v3 structure (baseline 200us -> v2 202us -> here):
  * Engine split per [128,4096] tile: DVE does <q,d>,<a,d> (STT+accum),
    ACT does |d|^2,|q|^2,|a|^2 (Square+accum) - both ~10-11.4us/tile
    under the 16.8us DMA cadence.
  * D_v1/D_v2 are folded into the dot-product STTs via the per-partition
    `scalar` operand, so each tile's contribution is dot*dv directly
    (no dots staging, one fewer mul per column).
  * `a` streams in halves for loop tiles and quarters for the last
    tile: after the final byte only ~1.2us of DVE work remains, not a
    4.4us full-width STT.  d/q stay whole-width.
  * Whole loop runs off the sqrt_and_others act table (Sqrt epilogue +
    Square norms, one hoisted load, no per-tile table thrash).  The
    BCE's Exp load has no data deps and hides in the collective wait.
  * Warm-up chain AllReduce[1] -> AllGather[1->8] -> AllToAll[8->8]
    during the stream: absorbs core-start skew, wakes ncfw, and logs
    per-algorithm collective cost into cc_ops each run.
"""

import os
import sys

import numpy as np

for _p in ("/opt/trn_rl_repo", "/root/.axon_site/_ro/trn_rl_repo"):
    if os.path.isdir(_p) and _p not in sys.path:
        sys.path.append(_p)

import concourse.bass as bass
import concourse.bacc as bacc
import concourse.mybir as mybir
import concourse.tile as tile
from concourse import bass_utils

N_CORES = 8
B, D = 8192, 4096
BS = B // N_CORES          # 1024 samples per core
P = 128                    # SBUF partitions
T = BS // P                # 8 tiles of 128 samples per core
EPS = 1e-6

f32 = mybir.dt.float32
Alu = mybir.AluOpType
Act = mybir.ActivationFunctionType

_CACHE = {}


def _build_program():
    nc = bacc.Bacc(
        "TRN2",
        target_bir_lowering=False,
        debug=False,
        num_devices=N_CORES,
    )

    samples = nc.dram_tensor("samples", [BS, 3, D], f32, kind="ExternalInput")
    labels = nc.dram_tensor("labels", [1], f32, kind="ExternalInput")
    dv1 = nc.dram_tensor("dv1", [BS], f32, kind="ExternalInput")
    dv2 = nc.dram_tensor("dv2", [BS], f32, kind="ExternalInput")
    out = nc.dram_tensor("out", [1, 1], f32, kind="ExternalOutput")

    with tile.TileContext(nc) as tc:
        with (
            tc.tile_pool(name="data", bufs=2) as data_pool,
            tc.tile_pool(name="junk", bufs=1) as junk_pool,
            tc.tile_pool(name="stats", bufs=1) as stats_pool,
            tc.tile_pool(name="psum", bufs=1, space="PSUM") as psum_pool,
            tc.tile_pool(name="dram", bufs=1, space="DRAM") as dram_pool,
        ):
            L = T - 1   # the last tile, streamed/computed out of line
            h2 = D // 2
            h4 = D // 4

            # --- Head of the DMA queue: last tile's d/q go first so
            # their heavy stats run in the warm-up window.
            dL = stats_pool.tile([P, D], f32, tag="dL")
            qL = stats_pool.tile([P, D], f32, tag="qL")
            nc.sync.dma_start(dL[:], samples[bass.ts(L, P), 2, :])
            nc.sync.dma_start(qL[:], samples[bass.ts(L, P), 0, :])

            # Interleaved stats columns: tile t owns columns 2t (q-side,
            # dv1-scaled) and 2t+1 (a-side, dv2-scaled).
            nprod = stats_pool.tile([P, 2 * T], f32, tag="nprod")
            inv = stats_pool.tile([P, 2 * T], f32, tag="inv")
            contrib = stats_pool.tile([P, 2 * T], f32, tag="contrib")

            # Small weight/label loads on SWDGE, off the HWDGE stream.
            dvb = stats_pool.tile([P, 2 * T], f32, tag="dvb")
            ltile = stats_pool.tile([1, 1], f32, tag="ltile")
            dvb_v = dvb[:].rearrange("p (t g) -> p t g", g=2)
            nc.gpsimd.dma_start(dvb_v[:, :, 0], dv1[:].rearrange("(n p) -> p n", p=P))
            nc.gpsimd.dma_start(dvb_v[:, :, 1], dv2[:].rearrange("(n p) -> p n", p=P))
            nc.gpsimd.dma_start(ltile[:], labels[None, :])
            lneg = stats_pool.tile([1, 1], f32, tag="lneg")
            nc.vector.tensor_scalar_mul(lneg[:], ltile[:], -1.0)

            ones = stats_pool.tile([P, 1], f32, tag="ones")
            nc.gpsimd.memset(ones[:], 1.0)

            # --- Warm-up collective chain (during the stream): absorbs
            # core-start skew, wakes ncfw, measures algorithms.
            warm = stats_pool.tile([1, 8], f32, tag="warm")
            nc.gpsimd.memset(warm[:], 0.0)
            w0_in = dram_pool.tile([1, 1], f32, tag="w0_in")
            w0_out = dram_pool.tile([1, 1], f32, tag="w0_out")
            w1_out = dram_pool.tile([8, 1], f32, tag="w1_out")
            w2_out = dram_pool.tile([8, 1], f32, tag="w2_out")
            nc.gpsimd.dma_start(w0_in[:], warm[0:1, 0:1])
            nc.gpsimd.collective_compute(
                "AllReduce", Alu.add,
                replica_groups=[list(range(N_CORES))],
                ins=[w0_in[:].opt()], outs=[w0_out[:].opt()],
            )
            nc.gpsimd.collective_compute(
                "AllGather", Alu.bypass,
                replica_groups=[list(range(N_CORES))],
                ins=[w0_out[:].opt()], outs=[w1_out[:].opt()],
            )
            nc.gpsimd.collective_compute(
                "AllToAll", Alu.bypass,
                replica_groups=[list(range(N_CORES))],
                ins=[w1_out[:].opt()], outs=[w2_out[:].opt()],
            )

            # --- Tile L's d/q-dependent stats in the warm-up window:
            # dd_L/qq_L (ACT), dv1-scaled qd_L (DVE), then the full
            # column-2L epilogue (none of it touches `a`).
            ddL = stats_pool.tile([P, 1], f32, tag="ddL")
            jga = junk_pool.tile([P, D], f32, tag="junk_act")
            nc.scalar.activation(
                out=jga[:], in_=dL[:], func=Act.Square, accum_out=ddL[:],
            )
            qdL = stats_pool.tile([P, 1], f32, tag="qdL")
            jdv = junk_pool.tile([P, D], f32, tag="junk_dve")
            cq = slice(2 * L, 2 * L + 1)
            nc.vector.scalar_tensor_tensor(
                out=jdv[:], in0=qL[:], scalar=dvb[:, cq], in1=dL[:],
                op0=Alu.mult, op1=Alu.mult, accum_out=qdL[:],
            )
            qqL = stats_pool.tile([P, 1], f32, tag="qqL")
            jga2 = junk_pool.tile([P, D], f32, tag="junk_act")
            nc.scalar.activation(
                out=jga2[:], in_=qL[:], func=Act.Square, accum_out=qqL[:],
            )
            nc.vector.tensor_mul(nprod[:, cq], qqL[:], ddL[:])
            nc.scalar.activation(inv[:, cq], nprod[:, cq], Act.Sqrt)
            nc.vector.tensor_scalar_max(inv[:, cq], inv[:, cq], EPS)
            nc.vector.reciprocal(inv[:, cq], inv[:, cq])
            nc.vector.tensor_mul(contrib[:, cq], qdL[:], inv[:, cq])

            # --- Main loop over tiles 0..T-2; `a` streams in halves.
            for t in range(T - 1):
                d_t = data_pool.tile([P, D], f32, tag="d")
                q_t = data_pool.tile([P, D], f32, tag="q")
                a_t = data_pool.tile([P, D], f32, tag="a")
                nc.sync.dma_start(d_t[:], samples[bass.ts(t, P), 2, :])
                nc.sync.dma_start(q_t[:], samples[bass.ts(t, P), 0, :])
                nc.sync.dma_start(a_t[:, 0:h2], samples[bass.ts(t, P), 1, 0:h2])
                nc.sync.dma_start(a_t[:, h2:D], samples[bass.ts(t, P), 1, h2:D])
                q, a, d = q_t[:], a_t[:], d_t[:]
                c0 = slice(2 * t, 2 * t + 1)
                c1 = slice(2 * t + 1, 2 * t + 2)
                c2 = slice(2 * t, 2 * t + 2)

                # ACT: dd, qq (whole), aa (halves)
                jg = junk_pool.tile([P, D], f32, tag="junk_act")
                dd = junk_pool.tile([P, 1], f32, tag="dd1")
                nc.scalar.activation(
                    out=jg[:], in_=d, func=Act.Square, accum_out=dd[:],
                )
                jq = junk_pool.tile([P, D], f32, tag="junk_act")
                qq = junk_pool.tile([P, 1], f32, tag="qq1")
                nc.scalar.activation(
                    out=jq[:], in_=q, func=Act.Square, accum_out=qq[:],
                )
                ja = junk_pool.tile([P, D], f32, tag="junk_act")
                aa_h = [
                    junk_pool.tile([P, 1], f32, tag=f"aa_h{i}", name=f"aa_h{i}")
                    for i in range(2)
                ]
                for i in range(2):
                    sl = slice(i * h2, (i + 1) * h2)
                    nc.scalar.activation(
                        out=ja[:, sl], in_=a[:, sl], func=Act.Square,
                        accum_out=aa_h[i][:],
                    )

                # DVE: dv1-scaled qd (whole), dv2-scaled ad (halves)
                jd = junk_pool.tile([P, D], f32, tag="junk_dve")
                qd = junk_pool.tile([P, 1], f32, tag="qd1")
                nc.vector.scalar_tensor_tensor(
                    out=jd[:], in0=q, scalar=dvb[:, c0], in1=d,
                    op0=Alu.mult, op1=Alu.mult, accum_out=qd[:],
                )
                jd2 = junk_pool.tile([P, D], f32, tag="junk_dve")
                ad_h = [
                    junk_pool.tile([P, 1], f32, tag=f"ad_h{i}", name=f"ad_h{i}")
                    for i in range(2)
                ]
                for i in range(2):
                    sl = slice(i * h2, (i + 1) * h2)
                    nc.vector.scalar_tensor_tensor(
                        out=jd2[:, sl], in0=a[:, sl], scalar=dvb[:, c1],
                        in1=d[:, sl],
                        op0=Alu.mult, op1=Alu.mult, accum_out=ad_h[i][:],
                    )
                ad = junk_pool.tile([P, 1], f32, tag="ad1")
                nc.vector.tensor_add(ad[:], ad_h[0][:], ad_h[1][:])
                aa = junk_pool.tile([P, 1], f32, tag="aa1")
                nc.vector.tensor_add(aa[:], aa_h[0][:], aa_h[1][:])

                # Epilogue: nprod, sqrt, clamp, reciprocal, contribution.
                nc.vector.tensor_mul(nprod[:, c0], qq[:], dd[:])
                nc.vector.tensor_mul(nprod[:, c1], aa[:], dd[:])
                nc.scalar.activation(inv[:, c2], nprod[:, c2], Act.Sqrt)
                nc.vector.tensor_scalar_max(inv[:, c2], inv[:, c2], EPS)
                nc.vector.reciprocal(inv[:, c2], inv[:, c2])
                nc.vector.tensor_mul(contrib[:, c0], qd[:], inv[:, c0])
                nc.vector.tensor_mul(contrib[:, c1], ad[:], inv[:, c1])

            # --- Tile L's `a` arrives last in quarters.
            aL = stats_pool.tile([P, D], f32, tag="aL")
            ad_run = stats_pool.tile([P, 1], f32, tag="ad_run")
            aa_run = stats_pool.tile([P, 1], f32, tag="aa_run")
            ad_q = [
                junk_pool.tile([P, 1], f32, tag=f"ad_q{i}", name=f"ad_q{i}")
                for i in range(4)
            ]
            aa_q = [
                junk_pool.tile([P, 1], f32, tag=f"aa_q{i}", name=f"aa_q{i}")
                for i in range(4)
            ]
            ca = slice(2 * L + 1, 2 * L + 2)
            jdL = junk_pool.tile([P, D], f32, tag="junk_dve")
            jaL = junk_pool.tile([P, D], f32, tag="junk_act")
            for i in range(4):
                sl = slice(i * h4, (i + 1) * h4)
                nc.sync.dma_start(aL[:, sl], samples[bass.ts(L, P), 1, sl])
                nc.vector.scalar_tensor_tensor(
                    out=jdL[:, sl], in0=aL[:, sl], scalar=dvb[:, ca],
                    in1=dL[:, sl],
                    op0=Alu.mult, op1=Alu.mult, accum_out=ad_q[i][:],
                )
                nc.scalar.activation(
                    out=jaL[:, sl], in_=aL[:, sl], func=Act.Square,
                    accum_out=aa_q[i][:],
                )
                if i == 1:
                    nc.vector.tensor_add(ad_run[:], ad_q[0][:], ad_q[1][:])
                    nc.vector.tensor_add(aa_run[:], aa_q[0][:], aa_q[1][:])
                elif i > 1:
                    nc.vector.tensor_add(ad_run[:], ad_run[:], ad_q[i][:])
                    nc.vector.tensor_add(aa_run[:], aa_run[:], aa_q[i][:])

            nc.vector.tensor_mul(nprod[:, ca], aa_run[:], ddL[:])
            nc.scalar.activation(inv[:, ca], nprod[:, ca], Act.Sqrt)
            nc.vector.tensor_scalar_max(inv[:, ca], inv[:, ca], EPS)
            nc.vector.reciprocal(inv[:, ca], inv[:, ca])
            nc.vector.tensor_mul(contrib[:, ca], ad_run[:], inv[:, ca])

            # Row-sum of all contributions; PE partition-reduce into an
            # [8,1] tile (all rows equal = this core's partial score).
            row_sum = stats_pool.tile([P, 1], f32, tag="row_sum")
            nc.vector.reduce_sum(row_sum[:], contrib[:], axis=mybir.AxisListType.X)
            zero8 = stats_pool.tile([P, N_CORES], f32, tag="zero8")
            nc.gpsimd.memset(zero8[:], 0.0)
            rs8 = stats_pool.tile([P, N_CORES], f32, tag="rs8")
            nc.vector.tensor_scalar_add(rs8[:], zero8[:], row_sum[:])
            psum_t = psum_pool.tile([N_CORES, 1], f32, tag="psum_s")
            nc.tensor.matmul(psum_t[:], rs8[:], ones[:], start=True, stop=True)
            partial8 = stats_pool.tile([N_CORES, 1], f32, tag="partial8")
            nc.vector.tensor_copy(partial8[:], psum_t[:])

            # AllToAll hands every rank all 8 partials in one Mesh phase
            # (~13us vs 23us for RDH ReduceScatter); sum them locally.
            cc_in = dram_pool.tile([N_CORES, 1], f32, tag="cc_in")
            cc_out = dram_pool.tile([N_CORES, 1], f32, tag="cc_out")
            nc.sync.dma_start(cc_in[:], partial8[:])
            nc.gpsimd.collective_compute(
                "AllToAll",
                Alu.bypass,
                replica_groups=[list(range(N_CORES))],
                ins=[cc_in[:].opt()],
                outs=[cc_out[:].opt()],
            )
            gath = stats_pool.tile([1, N_CORES], f32, tag="gath")
            nc.sync.dma_start(gath[:], cc_out[:].rearrange("a b -> b a"))
            red = stats_pool.tile([1, 1], f32, tag="red")
            nc.vector.reduce_sum(red[:], gath[:], axis=mybir.AxisListType.X)
            s = red[0:1, 0:1]

            # BCE with logits: ln(1+e^s) - s*y   (|s| = O(10) here).
            exp_t = stats_pool.tile([1, 1], f32, tag="exp_t")
            sp_t = stats_pool.tile([1, 1], f32, tag="sp_t")
            bce_t = stats_pool.tile([1, 1], f32, tag="bce_t")
            nc.scalar.activation(exp_t[:], s, Act.Exp)
            nc.scalar.activation(sp_t[:], exp_t[:], Act.Ln, bias=1.0)
            nc.vector.scalar_tensor_tensor(
                out=bce_t[:], in0=s, scalar=lneg[:], in1=sp_t[:],
                op0=Alu.mult, op1=Alu.add,
            )

            nc.sync.dma_start(out[:], bce_t[:])

    nc.compile()
    return nc


def _get_program():
    if "nc" not in _CACHE:
        _CACHE["nc"] = _build_program()
    return _CACHE["nc"]


def kernel(samples, labels, D_v1, D_v2):
    samples = np.asarray(samples, dtype=np.float32)
    labels = np.asarray(labels, dtype=np.float32)
    D_v1 = np.asarray(D_v1, dtype=np.float32)
    D_v2 = np.asarray(D_v2, dtype=np.float32)
    assert samples.shape == (B, 3, D), samples.shape

    nc = _get_program()

    in_maps = []
    for c in range(N_CORES):
        sl = slice(c * BS, (c + 1) * BS)
        in_maps.append(
            {
                "samples": np.ascontiguousarray(samples[sl]),
                "labels": labels,
                "dv1": np.ascontiguousarray(D_v1[sl]),
                "dv2": np.ascontiguousarray(D_v2[sl]),
            }
        )

    _tc = os.environ.get("KERNEL_TRACE_CORES")
    _kw = {"trace_cores": [int(x) for x in _tc.split(",")]} if _tc else {}
    try:
        res = bass_utils.run_bass_kernel_spmd(
            nc, in_maps, core_ids=list(range(N_CORES)), **_kw
        )
    except Exception:
        # A previously-wedged NeuronCore surfaces as an unrecoverable
        # exec error on the first attempt; the runtime resets it, so a
        # single retry recovers.
        res = bass_utils.run_bass_kernel_spmd(
            nc, in_maps, core_ids=list(range(N_CORES)), **_kw
        )
    _CACHE["last_results"] = res
    return np.asarray(res.results[0]["out"], dtype=np.float32).reshape(())


# revision 15
# speedup vs baseline: 1.2761x; 1.2761x over previous
"""Trainium2 Bass kernel for nn_Discriminator_15668040696127.

Computes:
    q, a, d = samples[:, 0], samples[:, 1], samples[:, 2]        # [B, D]
    cos1 = <q,d> / max(||q||*||d||, 1e-6)                         # [B]
    cos2 = <a,d> / max(||a||*||d||, 1e-6)                         # [B]
    score = cos1 @ D_v1 + cos2 @ D_v2                             # scalar
    out = BCE_with_logits(score, labels[0])                       # scalar

Sharding: data-parallel over B across 8 NeuronCores (1024 samples each).
Each core computes a partial score, broadcast into an [8,1] tile; an
AllToAll (Mesh, ~13us vs 23us for the RDH ReduceScatter) hands every
rank all 8 partials, which it sums locally before the scalar BCE; the
host reads core 0's output.

v3 structure (baseline 200us -> v2 202us -> here):
  * Engine split per [128,4096] tile: DVE does <q,d>,<a,d> (STT+accum),
    ACT does |d|^2,|q|^2,|a|^2 (Square+accum) - both ~10-11.4us/tile
    under the 16.8us DMA cadence.
  * D_v1/D_v2 are folded into the dot-product STTs via the per-partition
    `scalar` operand, so each tile's contribution is dot*dv directly
    (no dots staging, one fewer mul per column).
  * `a` streams in halves for loop tiles and quarters for the last
    tile: after the final byte only ~1.2us of DVE work remains, not a
    4.4us full-width STT.  d/q stay whole-width.
  * Whole loop runs off the sqrt_and_others act table (Sqrt epilogue +
    Square norms, one hoisted load, no per-tile table thrash).  The
    BCE's Exp load has no data deps and hides in the collective wait.
  * Warm-up chain AllReduce[1] -> AllGather[1->8] -> AllToAll[8->8]
    during the stream: absorbs core-start skew, wakes ncfw, and logs
    per-algorithm collective cost into cc_ops each run.
"""

import os
import sys

import numpy as np

for _p in ("/opt/trn_rl_repo", "/root/.axon_site/_ro/trn_rl_repo"):
    if os.path.isdir(_p) and _p not in sys.path:
        sys.path.append(_p)

import concourse.bass as bass
import concourse.bacc as bacc
import concourse.mybir as mybir
import concourse.tile as tile
from concourse import bass_utils

N_CORES = 8
B, D = 8192, 4096
BS = B // N_CORES          # 1024 samples per core
P = 128                    # SBUF partitions
T = BS // P                # 8 tiles of 128 samples per core
EPS = 1e-6

f32 = mybir.dt.float32
Alu = mybir.AluOpType
Act = mybir.ActivationFunctionType

_CACHE = {}


def _build_program():
    nc = bacc.Bacc(
        "TRN2",
        target_bir_lowering=False,
        debug=False,
        num_devices=N_CORES,
    )

    samples = nc.dram_tensor("samples", [BS, 3, D], f32, kind="ExternalInput")
    labels = nc.dram_tensor("labels", [1], f32, kind="ExternalInput")
    dv1 = nc.dram_tensor("dv1", [BS], f32, kind="ExternalInput")
    dv2 = nc.dram_tensor("dv2", [BS], f32, kind="ExternalInput")
    out = nc.dram_tensor("out", [1, 1], f32, kind="ExternalOutput")

    with tile.TileContext(nc) as tc:
        with (
            tc.tile_pool(name="data", bufs=2) as data_pool,
            tc.tile_pool(name="junk", bufs=1) as junk_pool,
            tc.tile_pool(name="stats", bufs=1) as stats_pool,
            tc.tile_pool(name="psum", bufs=1, space="PSUM") as psum_pool,
            tc.tile_pool(name="dram", bufs=1, space="DRAM") as dram_pool,
        ):
            L = T - 1   # the last tile, streamed/computed out of line
            h2 = D // 2
            h4 = D // 4

            # --- Head of the DMA queue: last tile's d/q go first so
            # their heavy stats run in the warm-up window.
            dL = stats_pool.tile([P, D], f32, tag="dL")
            qL = stats_pool.tile([P, D], f32, tag="qL")
            nc.sync.dma_start(dL[:], samples[bass.ts(L, P), 2, :])
            nc.sync.dma_start(qL[:], samples[bass.ts(L, P), 0, :])

            # First gpsimd-queue instruction: the warm-up collective
            # trigger (see comment below at its tile definitions).
            w0_in = dram_pool.tile([1, 1], f32, tag="w0_in")
            w0_out = dram_pool.tile([1, 1], f32, tag="w0_out")
            nc.gpsimd.collective_compute(
                "AllReduce", Alu.add,
                replica_groups=[list(range(N_CORES))],
                ins=[w0_in[:].opt()], outs=[w0_out[:].opt()],
            )

            # Interleaved stats columns: tile t owns columns 2t (q-side,
            # dv1-scaled) and 2t+1 (a-side, dv2-scaled).
            nprod = stats_pool.tile([P, 2 * T], f32, tag="nprod")
            inv = stats_pool.tile([P, 2 * T], f32, tag="inv")
            contrib = stats_pool.tile([P, 2 * T], f32, tag="contrib")

            # Small weight/label loads on SWDGE, off the HWDGE stream.
            dvb = stats_pool.tile([P, 2 * T], f32, tag="dvb")
            ltile = stats_pool.tile([1, 1], f32, tag="ltile")
            dvb_v = dvb[:].rearrange("p (t g) -> p t g", g=2)
            nc.gpsimd.dma_start(dvb_v[:, :, 0], dv1[:].rearrange("(n p) -> p n", p=P))
            nc.gpsimd.dma_start(dvb_v[:, :, 1], dv2[:].rearrange("(n p) -> p n", p=P))
            nc.gpsimd.dma_start(ltile[:], labels[None, :])
            lneg = stats_pool.tile([1, 1], f32, tag="lneg")
            nc.vector.tensor_scalar_mul(lneg[:], ltile[:], -1.0)

            ones = stats_pool.tile([P, 1], f32, tag="ones")
            nc.gpsimd.memset(ones[:], 1.0)

            # (The single warm-up collective above has no input deps -
            # the value is never read, uninitialized DRAM is fine - so
            # it triggers right after the preamble and the 60-120us ncfw
            # cold-start burns off during the stream.  Chained warm-ups
            # are a trap: they queue ahead of the real tail collective
            # on the cc stream and under cold-start variance push it
            # out by 50us+.)

            # --- Tile L's d/q-dependent stats in the warm-up window:
            # dd_L/qq_L (ACT), dv1-scaled qd_L (DVE), then the full
            # column-2L epilogue (none of it touches `a`).
            ddL = stats_pool.tile([P, 1], f32, tag="ddL")
            jga = junk_pool.tile([P, D], f32, tag="junk_act")
            nc.scalar.activation(
                out=jga[:], in_=dL[:], func=Act.Square, accum_out=ddL[:],
            )
            qdL = stats_pool.tile([P, 1], f32, tag="qdL")
            jdv = junk_pool.tile([P, D], f32, tag="junk_dve")
            cq = slice(2 * L, 2 * L + 1)
            nc.vector.scalar_tensor_tensor(
                out=jdv[:], in0=qL[:], scalar=dvb[:, cq], in1=dL[:],
                op0=Alu.mult, op1=Alu.mult, accum_out=qdL[:],
            )
            qqL = stats_pool.tile([P, 1], f32, tag="qqL")
            jga2 = junk_pool.tile([P, D], f32, tag="junk_act")
            nc.scalar.activation(
                out=jga2[:], in_=qL[:], func=Act.Square, accum_out=qqL[:],
            )
            nc.vector.tensor_mul(nprod[:, cq], qqL[:], ddL[:])
            nc.scalar.activation(inv[:, cq], nprod[:, cq], Act.Sqrt)
            nc.vector.tensor_scalar_max(inv[:, cq], inv[:, cq], EPS)
            nc.vector.reciprocal(inv[:, cq], inv[:, cq])
            nc.vector.tensor_mul(contrib[:, cq], qdL[:], inv[:, cq])

            # --- Main loop over tiles 0..T-2; `a` streams in halves.
            for t in range(T - 1):
                d_t = data_pool.tile([P, D], f32, tag="d")
                q_t = data_pool.tile([P, D], f32, tag="q")
                a_t = data_pool.tile([P, D], f32, tag="a")
                nc.sync.dma_start(d_t[:], samples[bass.ts(t, P), 2, :])
                nc.sync.dma_start(q_t[:], samples[bass.ts(t, P), 0, :])
                nc.sync.dma_start(a_t[:, 0:h2], samples[bass.ts(t, P), 1, 0:h2])
                nc.sync.dma_start(a_t[:, h2:D], samples[bass.ts(t, P), 1, h2:D])
                q, a, d = q_t[:], a_t[:], d_t[:]
                c0 = slice(2 * t, 2 * t + 1)
                c1 = slice(2 * t + 1, 2 * t + 2)
                c2 = slice(2 * t, 2 * t + 2)

                # ACT: dd, qq (whole), aa (halves)
                jg = junk_pool.tile([P, D], f32, tag="junk_act")
                dd = junk_pool.tile([P, 1], f32, tag="dd1")
                nc.scalar.activation(
                    out=jg[:], in_=d, func=Act.Square, accum_out=dd[:],
                )
                jq = junk_pool.tile([P, D], f32, tag="junk_act")
                qq = junk_pool.tile([P, 1], f32, tag="qq1")
                nc.scalar.activation(
                    out=jq[:], in_=q, func=Act.Square, accum_out=qq[:],
                )
                ja = junk_pool.tile([P, D], f32, tag="junk_act")
                aa_h = [
                    junk_pool.tile([P, 1], f32, tag=f"aa_h{i}", name=f"aa_h{i}")
                    for i in range(2)
                ]
                for i in range(2):
                    sl = slice(i * h2, (i + 1) * h2)
                    nc.scalar.activation(
                        out=ja[:, sl], in_=a[:, sl], func=Act.Square,
                        accum_out=aa_h[i][:],
                    )

                # DVE: dv1-scaled qd (whole), dv2-scaled ad (halves)
                jd = junk_pool.tile([P, D], f32, tag="junk_dve")
                qd = junk_pool.tile([P, 1], f32, tag="qd1")
                nc.vector.scalar_tensor_tensor(
                    out=jd[:], in0=q, scalar=dvb[:, c0], in1=d,
                    op0=Alu.mult, op1=Alu.mult, accum_out=qd[:],
                )
                jd2 = junk_pool.tile([P, D], f32, tag="junk_dve")
                ad_h = [
                    junk_pool.tile([P, 1], f32, tag=f"ad_h{i}", name=f"ad_h{i}")
                    for i in range(2)
                ]
                for i in range(2):
                    sl = slice(i * h2, (i + 1) * h2)
                    nc.vector.scalar_tensor_tensor(
                        out=jd2[:, sl], in0=a[:, sl], scalar=dvb[:, c1],
                        in1=d[:, sl],
                        op0=Alu.mult, op1=Alu.mult, accum_out=ad_h[i][:],
                    )
                ad = junk_pool.tile([P, 1], f32, tag="ad1")
                nc.vector.tensor_add(ad[:], ad_h[0][:], ad_h[1][:])
                aa = junk_pool.tile([P, 1], f32, tag="aa1")
                nc.vector.tensor_add(aa[:], aa_h[0][:], aa_h[1][:])

                # Epilogue: nprod, sqrt, clamp, reciprocal, contribution.
                nc.vector.tensor_mul(nprod[:, c0], qq[:], dd[:])
                nc.vector.tensor_mul(nprod[:, c1], aa[:], dd[:])
                nc.scalar.activation(inv[:, c2], nprod[:, c2], Act.Sqrt)
                nc.vector.tensor_scalar_max(inv[:, c2], inv[:, c2], EPS)
                nc.vector.reciprocal(inv[:, c2], inv[:, c2])
                nc.vector.tensor_mul(contrib[:, c0], qd[:], inv[:, c0])
                nc.vector.tensor_mul(contrib[:, c1], ad[:], inv[:, c1])

            # --- Tile L's `a` arrives last in quarters.
            aL = stats_pool.tile([P, D], f32, tag="aL")
            ad_run = stats_pool.tile([P, 1], f32, tag="ad_run")
            aa_run = stats_pool.tile([P, 1], f32, tag="aa_run")
            ad_q = [
                junk_pool.tile([P, 1], f32, tag=f"ad_q{i}", name=f"ad_q{i}")
                for i in range(4)
            ]
            aa_q = [
                junk_pool.tile([P, 1], f32, tag=f"aa_q{i}", name=f"aa_q{i}")
                for i in range(4)
            ]
            ca = slice(2 * L + 1, 2 * L + 2)
            jdL = junk_pool.tile([P, D], f32, tag="junk_dve")
            jaL = junk_pool.tile([P, D], f32, tag="junk_act")
            for i in range(4):
                sl = slice(i * h4, (i + 1) * h4)
                nc.sync.dma_start(aL[:, sl], samples[bass.ts(L, P), 1, sl])
                nc.vector.scalar_tensor_tensor(
                    out=jdL[:, sl], in0=aL[:, sl], scalar=dvb[:, ca],
                    in1=dL[:, sl],
                    op0=Alu.mult, op1=Alu.mult, accum_out=ad_q[i][:],
                )
                nc.scalar.activation(
                    out=jaL[:, sl], in_=aL[:, sl], func=Act.Square,
                    accum_out=aa_q[i][:],
                )
                if i == 1:
                    nc.vector.tensor_add(ad_run[:], ad_q[0][:], ad_q[1][:])
                    nc.vector.tensor_add(aa_run[:], aa_q[0][:], aa_q[1][:])
                elif i > 1:
                    nc.vector.tensor_add(ad_run[:], ad_run[:], ad_q[i][:])
                    nc.vector.tensor_add(aa_run[:], aa_run[:], aa_q[i][:])

            nc.vector.tensor_mul(nprod[:, ca], aa_run[:], ddL[:])
            nc.scalar.activation(inv[:, ca], nprod[:, ca], Act.Sqrt)
            nc.vector.tensor_scalar_max(inv[:, ca], inv[:, ca], EPS)
            nc.vector.reciprocal(inv[:, ca], inv[:, ca])
            nc.vector.tensor_mul(contrib[:, ca], ad_run[:], inv[:, ca])

            # Row-sum of all contributions; PE partition-reduce into an
            # [8,1] tile (all rows equal = this core's partial score).
            row_sum = stats_pool.tile([P, 1], f32, tag="row_sum")
            nc.vector.reduce_sum(row_sum[:], contrib[:], axis=mybir.AxisListType.X)
            zero8 = stats_pool.tile([P, N_CORES], f32, tag="zero8")
            nc.gpsimd.memset(zero8[:], 0.0)
            rs8 = stats_pool.tile([P, N_CORES], f32, tag="rs8")
            nc.vector.tensor_scalar_add(rs8[:], zero8[:], row_sum[:])
            psum_t = psum_pool.tile([N_CORES, 1], f32, tag="psum_s")
            nc.tensor.matmul(psum_t[:], rs8[:], ones[:], start=True, stop=True)
            partial8 = stats_pool.tile([N_CORES, 1], f32, tag="partial8")
            nc.vector.tensor_copy(partial8[:], psum_t[:])

            # AllToAll hands every rank all 8 partials in one Mesh phase
            # (~13us vs 23us for RDH ReduceScatter); sum them locally.
            cc_in = dram_pool.tile([N_CORES, 1], f32, tag="cc_in")
            cc_out = dram_pool.tile([N_CORES, 1], f32, tag="cc_out")
            nc.sync.dma_start(cc_in[:], partial8[:])
            nc.gpsimd.collective_compute(
                "AllToAll",
                Alu.bypass,
                replica_groups=[list(range(N_CORES))],
                ins=[cc_in[:].opt()],
                outs=[cc_out[:].opt()],
            )
            gath = stats_pool.tile([1, N_CORES], f32, tag="gath")
            nc.sync.dma_start(gath[:], cc_out[:].rearrange("a b -> b a"))
            red = stats_pool.tile([1, 1], f32, tag="red")
            nc.vector.reduce_sum(red[:], gath[:], axis=mybir.AxisListType.X)
            s = red[0:1, 0:1]

            # BCE with logits: ln(1+e^s) - s*y   (|s| = O(10) here).
            exp_t = stats_pool.tile([1, 1], f32, tag="exp_t")
            sp_t = stats_pool.tile([1, 1], f32, tag="sp_t")
            bce_t = stats_pool.tile([1, 1], f32, tag="bce_t")
            nc.scalar.activation(exp_t[:], s, Act.Exp)
            nc.scalar.activation(sp_t[:], exp_t[:], Act.Ln, bias=1.0)
            nc.vector.scalar_tensor_tensor(
                out=bce_t[:], in0=s, scalar=lneg[:], in1=sp_t[:],
                op0=Alu.mult, op1=Alu.add,
            )

            nc.sync.dma_start(out[:], bce_t[:])

    nc.compile()
    return nc


def _get_program():
    if "nc" not in _CACHE:
        _CACHE["nc"] = _build_program()
    return _CACHE["nc"]


def kernel(samples, labels, D_v1, D_v2):
    samples = np.asarray(samples, dtype=np.float32)
    labels = np.asarray(labels, dtype=np.float32)
    D_v1 = np.asarray(D_v1, dtype=np.float32)
    D_v2 = np.asarray(D_v2, dtype=np.float32)
    assert samples.shape == (B, 3, D), samples.shape

    nc = _get_program()

    in_maps = []
    for c in range(N_CORES):
        sl = slice(c * BS, (c + 1) * BS)
        in_maps.append(
            {
                "samples": np.ascontiguousarray(samples[sl]),
                "labels": labels,
                "dv1": np.ascontiguousarray(D_v1[sl]),
                "dv2": np.ascontiguousarray(D_v2[sl]),
            }
        )

    _tc = os.environ.get("KERNEL_TRACE_CORES")
    _kw = {"trace_cores": [int(x) for x in _tc.split(",")]} if _tc else {}
    try:
        res = bass_utils.run_bass_kernel_spmd(
            nc, in_maps, core_ids=list(range(N_CORES)), **_kw
        )
    except Exception:
        # A previously-wedged NeuronCore surfaces as an unrecoverable
        # exec error on the first attempt; the runtime resets it, so a
        # single retry recovers.
        res = bass_utils.run_bass_kernel_spmd(
            nc, in_maps, core_ids=list(range(N_CORES)), **_kw
        )
    _CACHE["last_results"] = res
    return np.asarray(res.results[0]["out"], dtype=np.float32).reshape(())
